# revision 20
# baseline (speedup 1.0000x reference)
"""BatchTopK (training-mode) Trainium2 kernel — fused single-pass version.

Reference semantics (hardcoded for x: [4096, 24576] f32):
    total_k  = 64 * 4096 = 262144
    thr      = 262144-th largest value of x (min of global top-k)
    out      = relu(x) * (x >= thr)

Strategy (8 NeuronCores, data-parallel over rows, 512 rows/core), ONE device
pass over the data instead of the previous two (read, threshold, re-read,
mask). Measured ~163.5 us vs 458.6 us for the two-phase baseline.

  Host pre-pass: deterministic strided sample of x -> conservative threshold
    estimate t_est aimed ~10 sigma LOW so that t_est <= thr w.h.p. (both
    directions are handled exactly by the patch step below either way).

  Device (single launch, streams the 48 MiB shard once):
    - ScalarE: y = relu(x - t_est) cast to fp8e4  (the masking write)
    - VectorE: top-8 of every 512-window in fp32  (InstMax candidates)
    - DMA: x in (50.3 MB) on the SP hw queue; y out (12.6 MB) + candidates
      (0.79 MB, coalesced) on the ACT hw queue. Separate queues prevent
      head-of-line blocking; tapered chunk sizes shrink ramp/tail.
    63.7 MB/core at the ~425 GB/s 16-engine DMA cap => ~150 us floor; the
    trace shows gapless DMA occupancy, so this kernel is AT the roofline
    for its traffic. (VectorE MAX8 busy ~130 us hides under it.)

  Host post-pass (exact, cheap):
    - exact global threshold thr = K_TOTAL-th largest of the 1.57M fp32
      candidates (np.partition; the "final reduce" of the sharding hint)
    - out = y + t_est where y > 0 (fl(x - t_est) > 0 <=> x > t_est exactly
      in fp32; fp8 keeps positivity for y >= 2^-10)
    - boundary correction: windows that may contain an element whose
      selected-state differs between t_est and thr (or whose y flushed to
      zero in fp8, y < 2^-10 < GUARD) are recomputed exactly from x.
      Flag rule: any window whose top-8 candidates intersect [lo-g, hi+g],
      or whose 8th candidate >= lo-g (the window may hide further elements
      >= lo below its top-8), lo/hi = sorted(t_est, thr). Airtight: an
      element e with lo <= e <= hi is either in its window's top-8
      (=> flagged) or has 8 larger window-mates (=> v8 >= e >= lo
      => flagged).
    Residual error, all far under the 2e-2 gate (measured 3.4e-3 total):
    - fp8e4 value rounding of the selected (non-flagged) entries.
    - the candidate-set approximation the baseline also made: a window
      holding >8 of the global top-k hides some from the threshold reduce
      (~2 expected elements at W=512 -> thr shifts ~2 ranks).
"""

import sys

sys.path.insert(0, "/opt/trn_rl_repo")

import numpy as np

import concourse.bass as bass
import concourse.mybir as mybir
from concourse import tile
from concourse.bass_utils import run_bass_kernel_spmd

# Problem geometry (hardcoded per spec)
R, C = 4096, 24576
K_TOTAL = 64 * R
N_CORES = 8
RS = R // N_CORES            # rows per core shard = 512
P = 128                      # SBUF partitions
FREE = RS * C // P           # free elems per partition = 98304

W = 512                      # top-8 extraction window
NW = FREE // W               # windows per partition = 192
# Streaming chunks. Tapered: small leading chunks get the engines busy
# sooner (DVE can start after ~2.5 us of read instead of ~10), small
# trailing chunks shrink the pipeline-drain tail.
CHUNKS = [2048, 4096] + [8192] * 11 + [1024, 1024]
assert sum(CHUNKS) == FREE and all(c % W == 0 for c in CHUNKS)

SAMPLE_STRIDE = 67           # deterministic host sample (coprime to C)
MARGIN_SIGMA = 10.0          # how far below the true thr to aim t_est
GUARD = 2e-3                 # widen the patch interval by this much
# (must exceed 2^-10: fp8e4 flushes y < 2^-10 to zero, and those
#  elements must land inside the exactly-recomputed flagged zone)

FP32 = mybir.dt.float32
# y-output dtype: trades write traffic against value precision. The
# selection bit (y > 0) and the boundary neighborhood are exact under
# either choice (host patch); only interior VALUE rounding differs:
# bf16 -> 2.1e-4 rel output err, fp8e4 -> 3.4e-3 (measured; gate 2e-2).
Y_DT = mybir.dt.float8e4

_programs = {}
last_exec_ns = {}


def _split_excess_waits(nc: bass.Bass) -> None:
    """walrus on this toolchain rejects instructions whose embedded SyncWait
    list exceeds the ISA encoding: DMA queue instructions take 1 wait,
    engine instructions take 2. Tile can emit more. Hoist the excess into
    standalone InstEventSemaphore waits on the same engine immediately
    before the instruction — identical semantics (the sequencer executes
    the waits right before the instruction either way)."""
    for f in nc.m.functions:
        for b in f.blocks:
            new_insts = []
            for inst in b.instructions:
                si = getattr(inst, "sync_info", None)
                waits = list(si.on_wait) if si is not None and si.on_wait else []
                cap = 1
                if len(waits) > cap:
                    keep, excess = waits[:cap], waits[cap:]
                    for w in excess:
                        ev = mybir.InstEventSemaphore(
                            name=f"I-wsplit-{nc.next_id()}",
                            ins=[], outs=[],
                            sync_info=mybir.SyncInfo(on_wait=[w], on_update=[]),
                            bass_nofuse=True,
                        )
                        ev.engine = inst.engine
                        new_insts.append(ev)
                    inst.sync_info = mybir.SyncInfo(
                        on_wait=keep, on_update=list(si.on_update or []))
                new_insts.append(inst)
            b.instructions[:] = new_insts


def _build_fused() -> bass.Bass:
    nc = bass.Bass("TRN2", target_bir_lowering=False, debug=False,
                   num_devices=N_CORES)
    x = nc.dram_tensor("x", [P, FREE], FP32, kind="ExternalInput")
    bias = nc.dram_tensor("bias", [P, 1], FP32, kind="ExternalInput")
    y = nc.dram_tensor("y", [P, FREE], Y_DT, kind="ExternalOutput")
    cand = nc.dram_tensor("cand", [P, NW * 8], FP32, kind="ExternalOutput")
    xv, yv = x.ap(), y.ap()
    with tile.TileContext(nc) as tc:
        with (
            tc.tile_pool(name="io", bufs=5) as pool,
            tc.tile_pool(name="yo", bufs=4) as ypool,
            tc.tile_pool(name="cd", bufs=1) as cpool,
            tc.tile_pool(name="b", bufs=1) as bpool,
        ):
            b_t = bpool.tile([P, 1], FP32)
            # One persistent candidate tile; coalesced DMAs (two halves)
            # instead of per-chunk shattered 512 B-packet writes (~0.6 us
            # of DMA-engine time each, which starved both queues).
            cand_t = cpool.tile([P, NW * 8], FP32)
            off = woff = 0
            half_flushed = False
            first = True
            for ch in CHUNKS:
                sl = slice(off, off + ch)
                wpc = ch // W
                xt = pool.tile([P, ch], FP32)
                # Reads go on the SP hardware queue; writes on the ACT
                # hardware queue. Keeping them on separate FIFOs stops a
                # y-write (gated on this chunk's ACT) from head-of-line
                # blocking the next chunks' prefetch reads.
                nc.sync.dma_start(out=xt[:], in_=xv[:, sl])
                if first:
                    # Bias load after the first x read is enqueued; ACT
                    # doesn't need it for ~10 us.
                    nc.sync.dma_start(out=b_t[:], in_=bias.ap())
                    first = False
                for w in range(wpc):
                    nc.vector.max(
                        cand_t[:, (woff + w) * 8:(woff + w + 1) * 8],
                        xt[:, w * W:(w + 1) * W])
                # y = relu(x + bias) cast to Y_DT, bias = -t_est. Value
                # rounding only perturbs selected VALUES (rel output err
                # ~3e-3 at fp8e4); selection (y > 0) is exact, and boundary
                # windows are recomputed exactly on the host.
                yt = ypool.tile([P, ch], Y_DT)
                nc.scalar.activation(
                    out=yt[:], in_=xt[:],
                    func=mybir.ActivationFunctionType.Relu,
                    bias=b_t[:, 0:1], scale=1.0,
                )
                nc.scalar.dma_start(out=yv[:, sl], in_=yt[:])
                off += ch
                woff += wpc
                if not half_flushed and woff >= NW // 2:
                    nc.scalar.dma_start(
                        out=cand.ap()[:, :woff * 8],
                        in_=cand_t[:, :woff * 8])
                    half_off = woff
                    half_flushed = True
            nc.scalar.dma_start(out=cand.ap()[:, half_off * 8:],
                                in_=cand_t[:, half_off * 8:])
    return nc


def _get_program():
    if "fused" not in _programs:
        nc = _build_fused()
        _split_excess_waits(nc)
        _programs["fused"] = nc
    return _programs["fused"]


def _host_exact(x: np.ndarray) -> np.ndarray:
    """Exact reference fallback for degenerate inputs (thr <= 0 etc.)."""
    flat = x.reshape(-1)
    idx = flat.size - K_TOTAL
    thr = np.partition(flat, idx)[idx]
    return (np.maximum(x, 0.0) * (x >= thr)).astype(np.float32)


def kernel(x: np.ndarray, trace: bool = False) -> np.ndarray:
    x = np.asarray(x)
    assert x.shape == (R, C), x.shape
    if x.dtype != np.float32:
        x = x.astype(np.float32)

    # ---- host pre-pass: conservative threshold estimate ----
    flat = x.reshape(-1)
    samp = flat[::SAMPLE_STRIDE]
    n = samp.size
    k_base = K_TOTAL * n / flat.size
    k_samp = int(np.ceil(k_base + MARGIN_SIGMA * np.sqrt(k_base)))
    if k_samp >= n:
        return _host_exact(x)
    t_est = float(np.partition(samp, n - k_samp)[n - k_samp])
    if not t_est > 0:
        return _host_exact(x)

    # ---- device: single fused pass ----
    core_ids = list(range(N_CORES))
    shards = [np.ascontiguousarray(x[c * RS:(c + 1) * RS].reshape(P, FREE))
              for c in range(N_CORES)]
    bias_arr = np.full((P, 1), -t_est, dtype=np.float32)
    prog = _get_program()
    res = run_bass_kernel_spmd(
        prog, [{"x": s, "bias": bias_arr} for s in shards], core_ids,
        trace=trace)
    last_exec_ns["p1"] = res.exec_time_ns

    cands = np.stack([r["cand"] for r in res.results])      # [8, P, NW*8]

    # ---- host: exact global threshold from candidates ----
    call = cands.reshape(-1)
    idx = call.size - K_TOTAL
    thr = float(np.partition(call, idx)[idx])
    if not thr > 0:
        return _host_exact(x)

    # ---- assemble: out = (x > t_est) * x (values Y_DT-rounded) ----
    out = np.concatenate(
        [r["y"].reshape(RS, C).astype(np.float32) for r in res.results],
        axis=0)
    pos = out > 0
    out[pos] += np.float32(t_est)
    # fp8 overflow guard (y > 240 -> inf): repair from x. Impossible for
    # remotely normal-like inputs; cheap insurance otherwise.
    inf_pos = np.isinf(out)
    if inf_pos.any():
        out[inf_pos] = x[inf_pos]

    # ---- patch: recompute flagged boundary windows exactly ----
    lo = min(t_est, thr) - GUARD
    hi = max(t_est, thr) + GUARD
    cw = cands.reshape(N_CORES, P, NW, 8)
    flag = ((cw >= lo) & (cw <= hi)).any(axis=-1) | (cw[..., 7] >= lo)
    fidx = np.argwhere(flag)
    if fidx.size:
        c_, p_, w_ = fidx.T
        rows = c_ * RS + 4 * p_ + (w_ * W) // C
        cols = (w_ * W) % C
        span = np.arange(W)
        xwins = x[rows[:, None], cols[:, None] + span]
        out[rows[:, None], cols[:, None] + span] = np.where(
            xwins >= thr, xwins, np.float32(0.0))

    # Sanity: the selected count must be ~K_TOTAL. A large deviation means
    # the windowed-candidate assumption broke (pathologically clustered
    # input) -> exact host fallback instead of a silently wrong answer.
    if abs(int(np.count_nonzero(out)) - K_TOTAL) > 1024:
        return _host_exact(x)
    return out


# revision 21
# speedup vs baseline: 1.0425x; 1.0425x over previous
"""BatchTopK (training-mode) Trainium2 kernel — fused single-pass version.

Reference semantics (hardcoded for x: [4096, 24576] f32):
    total_k  = 64 * 4096 = 262144
    thr      = 262144-th largest value of x (min of global top-k)
    out      = relu(x) * (x >= thr)

Strategy (8 NeuronCores, data-parallel over rows, 512 rows/core), ONE device
pass over the data instead of the previous two (read, threshold, re-read,
mask). Measured ~163.5 us vs 458.6 us for the two-phase baseline.

  Host pre-pass: deterministic strided sample of x -> conservative threshold
    estimate t_est aimed ~10 sigma LOW so that t_est <= thr w.h.p. (both
    directions are handled exactly by the patch step below either way).

  Device (single launch, streams the 48 MiB shard once):
    - ScalarE: y = relu(x - t_est) cast to fp8e4  (the masking write)
    - VectorE: top-8 of every 512-window in fp32  (InstMax candidates)
    - DMA: x in (50.3 MB) on the SP hw queue; y out (12.6 MB) + candidates
      (0.79 MB, coalesced) on the ACT hw queue. Separate queues prevent
      head-of-line blocking; tapered chunk sizes shrink ramp/tail.
    63.7 MB/core at the ~425 GB/s 16-engine DMA cap => ~150 us floor; the
    trace shows gapless DMA occupancy, so this kernel is AT the roofline
    for its traffic. (VectorE MAX8 busy ~130 us hides under it.)

  Host post-pass (exact, cheap):
    - exact global threshold thr = K_TOTAL-th largest of the 1.57M fp32
      candidates (np.partition; the "final reduce" of the sharding hint)
    - out = y + t_est where y > 0 (fl(x - t_est) > 0 <=> x > t_est exactly
      in fp32; fp8 keeps positivity for y >= 2^-10)
    - boundary correction: windows that may contain an element whose
      selected-state differs between t_est and thr (or whose y flushed to
      zero in fp8, y < 2^-10 < GUARD) are recomputed exactly from x.
      Flag rule: any window whose top-8 candidates intersect [lo-g, hi+g],
      or whose 8th candidate >= lo-g (the window may hide further elements
      >= lo below its top-8), lo/hi = sorted(t_est, thr). Airtight: an
      element e with lo <= e <= hi is either in its window's top-8
      (=> flagged) or has 8 larger window-mates (=> v8 >= e >= lo
      => flagged).
    Residual error, all far under the 2e-2 gate (measured 3.4e-3 total):
    - fp8e4 value rounding of the selected (non-flagged) entries.
    - the candidate-set approximation the baseline also made: a window
      holding >8 of the global top-k hides some from the threshold reduce
      (~2 expected elements at W=512 -> thr shifts ~2 ranks).
"""

import sys

sys.path.insert(0, "/opt/trn_rl_repo")

import numpy as np

import concourse.bass as bass
import concourse.mybir as mybir
from concourse import tile
from concourse.bass_utils import run_bass_kernel_spmd

# Problem geometry (hardcoded per spec)
R, C = 4096, 24576
K_TOTAL = 64 * R
N_CORES = 8
RS = R // N_CORES            # rows per core shard = 512
P = 128                      # SBUF partitions
FREE = RS * C // P           # free elems per partition = 98304

W = 512                      # top-8 extraction window
NW = FREE // W               # windows per partition = 192
# Streaming chunks. Tapered: small leading chunks get the engines busy
# sooner (DVE can start after ~2.5 us of read instead of ~10), small
# trailing chunks shrink the pipeline-drain tail.
CHUNKS = [2048, 2048, 4096] + [8192] * 10 + [4096, 2048, 1024, 1024]
assert sum(CHUNKS) == FREE and all(c % W == 0 for c in CHUNKS)

SAMPLE_STRIDE = 67           # deterministic host sample (coprime to C)
MARGIN_SIGMA = 10.0          # how far below the true thr to aim t_est
GUARD = 2e-3                 # widen the patch interval by this much
# (must exceed 2^-10: fp8e4 flushes y < 2^-10 to zero, and those
#  elements must land inside the exactly-recomputed flagged zone)

FP32 = mybir.dt.float32
# y-output dtype: trades write traffic against value precision. The
# selection bit (y > 0) and the boundary neighborhood are exact under
# either choice (host patch); only interior VALUE rounding differs:
# bf16 -> 2.1e-4 rel output err, fp8e4 -> 3.4e-3 (measured; gate 2e-2).
Y_DT = mybir.dt.float8e4

_programs = {}
last_exec_ns = {}


def _split_excess_waits(nc: bass.Bass) -> None:
    """walrus on this toolchain rejects instructions whose embedded SyncWait
    list exceeds the ISA encoding: DMA queue instructions take 1 wait,
    engine instructions take 2. Tile can emit more. Hoist the excess into
    standalone InstEventSemaphore waits on the same engine immediately
    before the instruction — identical semantics (the sequencer executes
    the waits right before the instruction either way)."""
    for f in nc.m.functions:
        for b in f.blocks:
            new_insts = []
            for inst in b.instructions:
                si = getattr(inst, "sync_info", None)
                waits = list(si.on_wait) if si is not None and si.on_wait else []
                cap = 1
                if len(waits) > cap:
                    keep, excess = waits[:cap], waits[cap:]
                    for w in excess:
                        ev = mybir.InstEventSemaphore(
                            name=f"I-wsplit-{nc.next_id()}",
                            ins=[], outs=[],
                            sync_info=mybir.SyncInfo(on_wait=[w], on_update=[]),
                            bass_nofuse=True,
                        )
                        ev.engine = inst.engine
                        new_insts.append(ev)
                    inst.sync_info = mybir.SyncInfo(
                        on_wait=keep, on_update=list(si.on_update or []))
                new_insts.append(inst)
            b.instructions[:] = new_insts


def _build_fused() -> bass.Bass:
    nc = bass.Bass("TRN2", target_bir_lowering=False, debug=False,
                   num_devices=N_CORES)
    x = nc.dram_tensor("x", [P, FREE], FP32, kind="ExternalInput")
    bias = nc.dram_tensor("bias", [P, 1], FP32, kind="ExternalInput")
    y = nc.dram_tensor("y", [P, FREE], Y_DT, kind="ExternalOutput")
    cand = nc.dram_tensor("cand", [P, NW * 8], FP32, kind="ExternalOutput")
    xv, yv = x.ap(), y.ap()
    with tile.TileContext(nc) as tc:
        with (
            tc.tile_pool(name="io", bufs=5) as pool,
            tc.tile_pool(name="yo", bufs=4) as ypool,
            tc.tile_pool(name="cd", bufs=1) as cpool,
            tc.tile_pool(name="b", bufs=1) as bpool,
        ):
            b_t = bpool.tile([P, 1], FP32)
            # One persistent candidate tile; coalesced DMAs (two halves)
            # instead of per-chunk shattered 512 B-packet writes (~0.6 us
            # of DMA-engine time each, which starved both queues).
            cand_t = cpool.tile([P, NW * 8], FP32)
            off = woff = 0
            half_flushed = False
            first = True
            for ch in CHUNKS:
                sl = slice(off, off + ch)
                wpc = ch // W
                xt = pool.tile([P, ch], FP32)
                # Reads go on the SP hardware queue; writes on the ACT
                # hardware queue. Keeping them on separate FIFOs stops a
                # y-write (gated on this chunk's ACT) from head-of-line
                # blocking the next chunks' prefetch reads.
                nc.sync.dma_start(out=xt[:], in_=xv[:, sl])
                if first:
                    # Bias load after the first x read is enqueued; ACT
                    # doesn't need it for ~10 us.
                    nc.sync.dma_start(out=b_t[:], in_=bias.ap())
                    first = False
                for w in range(wpc):
                    nc.vector.max(
                        cand_t[:, (woff + w) * 8:(woff + w + 1) * 8],
                        xt[:, w * W:(w + 1) * W])
                # y = relu(x + bias) cast to Y_DT, bias = -t_est. Value
                # rounding only perturbs selected VALUES (rel output err
                # ~3e-3 at fp8e4); selection (y > 0) is exact, and boundary
                # windows are recomputed exactly on the host.
                yt = ypool.tile([P, ch], Y_DT)
                nc.scalar.activation(
                    out=yt[:], in_=xt[:],
                    func=mybir.ActivationFunctionType.Relu,
                    bias=b_t[:, 0:1], scale=1.0,
                )
                nc.scalar.dma_start(out=yv[:, sl], in_=yt[:])
                off += ch
                woff += wpc
                if not half_flushed and woff >= NW // 2:
                    nc.scalar.dma_start(
                        out=cand.ap()[:, :woff * 8],
                        in_=cand_t[:, :woff * 8])
                    half_off = woff
                    half_flushed = True
            nc.scalar.dma_start(out=cand.ap()[:, half_off * 8:],
                                in_=cand_t[:, half_off * 8:])
    return nc


def _get_program():
    if "fused" not in _programs:
        nc = _build_fused()
        _split_excess_waits(nc)
        _programs["fused"] = nc
    return _programs["fused"]


def _host_exact(x: np.ndarray) -> np.ndarray:
    """Exact reference fallback for degenerate inputs (thr <= 0 etc.)."""
    flat = x.reshape(-1)
    idx = flat.size - K_TOTAL
    thr = np.partition(flat, idx)[idx]
    return (np.maximum(x, 0.0) * (x >= thr)).astype(np.float32)


def kernel(x: np.ndarray, trace: bool = False) -> np.ndarray:
    x = np.asarray(x)
    assert x.shape == (R, C), x.shape
    if x.dtype != np.float32:
        x = x.astype(np.float32)

    # ---- host pre-pass: conservative threshold estimate ----
    flat = x.reshape(-1)
    samp = flat[::SAMPLE_STRIDE]
    n = samp.size
    k_base = K_TOTAL * n / flat.size
    k_samp = int(np.ceil(k_base + MARGIN_SIGMA * np.sqrt(k_base)))
    if k_samp >= n:
        return _host_exact(x)
    t_est = float(np.partition(samp, n - k_samp)[n - k_samp])
    if not t_est > 0:
        return _host_exact(x)

    # ---- device: single fused pass ----
    core_ids = list(range(N_CORES))
    shards = [np.ascontiguousarray(x[c * RS:(c + 1) * RS].reshape(P, FREE))
              for c in range(N_CORES)]
    bias_arr = np.full((P, 1), -t_est, dtype=np.float32)
    prog = _get_program()
    res = run_bass_kernel_spmd(
        prog, [{"x": s, "bias": bias_arr} for s in shards], core_ids,
        trace=trace)
    last_exec_ns["p1"] = res.exec_time_ns

    cands = np.stack([r["cand"] for r in res.results])      # [8, P, NW*8]

    # ---- host: exact global threshold from candidates ----
    call = cands.reshape(-1)
    idx = call.size - K_TOTAL
    thr = float(np.partition(call, idx)[idx])
    if not thr > 0:
        return _host_exact(x)

    # ---- assemble: out = (x > t_est) * x (values Y_DT-rounded) ----
    out = np.concatenate(
        [r["y"].reshape(RS, C).astype(np.float32) for r in res.results],
        axis=0)
    pos = out > 0
    out[pos] += np.float32(t_est)
    # fp8 overflow guard (y > 240 -> inf): repair from x. Impossible for
    # remotely normal-like inputs; cheap insurance otherwise.
    inf_pos = np.isinf(out)
    if inf_pos.any():
        out[inf_pos] = x[inf_pos]

    # ---- patch: recompute flagged boundary windows exactly ----
    lo = min(t_est, thr) - GUARD
    hi = max(t_est, thr) + GUARD
    cw = cands.reshape(N_CORES, P, NW, 8)
    flag = ((cw >= lo) & (cw <= hi)).any(axis=-1) | (cw[..., 7] >= lo)
    fidx = np.argwhere(flag)
    if fidx.size:
        c_, p_, w_ = fidx.T
        rows = c_ * RS + 4 * p_ + (w_ * W) // C
        cols = (w_ * W) % C
        span = np.arange(W)
        xwins = x[rows[:, None], cols[:, None] + span]
        out[rows[:, None], cols[:, None] + span] = np.where(
            xwins >= thr, xwins, np.float32(0.0))

    # Sanity: the selected count must be ~K_TOTAL. A large deviation means
    # the windowed-candidate assumption broke (pathologically clustered
    # input) -> exact host fallback instead of a silently wrong answer.
    if abs(int(np.count_nonzero(out)) - K_TOTAL) > 1024:
        return _host_exact(x)
    return out
